# revision 17
# baseline (speedup 1.0000x reference)
# Multi-head attention (B=2, S=2048, D=1024, H=16, head_dim=64) with bool mask,
# sharded across 8 TRN2 NeuronCores: core c -> batch c//4, heads 4*(c%4)..4*(c%4)+3.
#
# Per-core device kernel (scores computed transposed: scoresT[k, q]):
#   scoresT = K @ Q^T                 (PE bf16, lhsT = K^T strip, rhs = Q^T)
#   atp     = exp(scoresT/8)          (ACT exp scale=1/8, psum -> psum bf16)
#   at      = atp * (1-m)T            (DVE mult, psum -> SBUF bf16)
#   out[q,d] += at_chunk^T @ [V|1]    (PE bf16: lhsT = at chunk (stationary),
#                                      rhs = V'[128,65]; col 64 accumulates Z)
#   out     = psO[:, :, 0:64] / Z     (DVE reciprocal + broadcast multiply)
#
# The AV matmul uses the attention chunk as the stationary operand so the
# output lands non-transposed ([q, d] with q on partitions): free size is 65
# instead of 512 per instruction (half the PE cycles of the V-stationary
# form) and the final PE transposes disappear entirely.
#
# Host side (inside kernel()): slice per-core shards, pre-transpose Q/K per
# head ([64, S] head-dim-major, bf16), pre-transpose the inverted mask to
# bf16, reassemble the 8 per-core bf16 outputs into the full f32 output.

import sys

import numpy as np

for _p in ("/opt/trn_rl_repo",):
    if _p not in sys.path:
        sys.path.insert(0, _p)

import ml_dtypes

import concourse.bass as bass  # noqa: F401  (engine types reachable via nc)
import concourse.tile as tile
from concourse import bacc, mybir
from concourse.bass_utils import run_bass_kernel_spmd

F32 = mybir.dt.float32
BF16 = mybir.dt.bfloat16

S = 2048          # sequence length
HD = 64           # head dim
HPC = 4           # heads per core
NCORES = 8
B = 2
H = 16
D = H * HD


def build_program(s=S, reps=1):
    """Build the single-core SPMD program. Returns the compiled Bacc object.

    reps>1 emits the whole body (loads+compute+stores) that many times in one
    NEFF — used to measure device time by wall-clock differencing."""
    nc = bacc.Bacc()

    KS = s // 128            # number of k strips
    QG = 1024 if s >= 1024 else s   # q group width (ACT/DVE instruction width)
    NQG = s // QG            # q groups
    NQC = max(QG // 512, 1)  # 512-wide matmul chunks per q group (psum bank)
    QC = min(512, QG)        # matmul chunk width
    NCH = QG // 128          # 128-wide q chunks per group (AV granularity)
    CPB = 4                  # psO chunks per 2KB psum bank (zero region)
    LAG = min(4, KS)         # AV strips emitted this many strips behind QK

    qkT_d = nc.declare_dram_parameter("qkT", [2, HPC * HD, s], BF16, isOutput=False)
    v_d = nc.declare_dram_parameter("v", [s, HPC * HD], BF16, isOutput=False)
    nmT_d = nc.declare_dram_parameter("nmT", [s, s], BF16, isOutput=False)
    out_d = nc.declare_dram_parameter("out", [s, HPC * HD], BF16, isOutput=True)

    # DRAM views with the k/q axis split into strips of 128 partitions
    nm_view = nmT_d[:].rearrange("(ks p) q -> p ks q", p=128)
    v_view = v_d[:].rearrange("(ks p) (h d) -> p ks h d", p=128, h=HPC)
    out_view = out_d[:].rearrange("(sq p) c -> p sq c", p=128)

    with tile.TileContext(nc) as tc:
        with (
            tc.tile_pool(name="const", bufs=1) as const,
            tc.tile_pool(name="wq", bufs=1) as wq,
            tc.tile_pool(name="attn", bufs=20) as apool,
            tc.tile_pool(name="stat", bufs=4) as spool,
            tc.tile_pool(name="oasm", bufs=1) as opool,
            tc.tile_pool(name="psS", bufs=2, space="PSUM") as psS_pool,
            tc.tile_pool(name="psO", bufs=2, space="PSUM") as psO_pool,
        ):
            # Preload the exp table (emitted before any real exp; runs while
            # the first DMAs stream).
            warm = const.tile([128, 1], F32)
            nc.vector.memset(warm, 0.0)
            nc.scalar.activation(warm, warm, mybir.ActivationFunctionType.Exp)

            # Warm the PE HAM clock gate while input DMAs run: ~3us of dummy
            # matmuls so the first real QKs run at 2.4GHz.
            zb = const.tile([128, 128], BF16)
            nc.vector.memset(zb, 0.0)
            for _ in range(24):
                wmm = psS_pool.tile([128, QG], F32, tag="psS")
                nc.tensor.matmul(
                    wmm[:, :128], lhsT=zb[0:64, :], rhs=zb[0:64, :],
                    start=True, stop=True,
                )

            def qk_src(pair):
                return qkT_d[:, 128 * pair:128 * pair + 128, :].rearrange(
                    "t p s -> p t s"
                )

            def emit_body():
                # Q^T / K^T head pairs: [128, s] (head 2p on partitions 0-63,
                # head 2p+1 on partitions 64-127).
                qks = []
                for pair in range(HPC // 2):
                    qk = wq.tile([128, 2, s], BF16, tag=f"qkT{pair}")
                    qks.append(qk)
                # V' staging: [128, ks, h, 65] with a ones column at 64 so the
                # AV matmul's 65th output column accumulates the softmax
                # denominator Z. V lands via interleaved DMA; the ones column
                # is memset once (disjoint subtile, no dependency on the DMA).
                vps = wq.tile([128, KS, HPC, HD + 1], BF16, tag="vps")
                nm_sb = wq.tile([128, KS, s], BF16, tag="nm")
                nc.vector.memset(vps[:, :, :, HD:HD + 1], 1.0)

                # DMA choreography (s=2048): two queues only — SP (nc.sync)
                # and Pool SWDGE (nc.gpsimd) — so the ACT and DVE sequencers
                # are never blocked behind a DMA wait. Ordered by first use:
                # K strips + first Q group first, mask halves interleaved,
                # V early (AV matmuls sit in the in-order PE queue).
                QH = QG  # nm half width
                if s == 2048:
                    A, Bq = nc.sync, nc.gpsimd
                    # The model's DMA device is effectively serial, so the
                    # ordering across the queues is what matters: the
                    # first-QK inputs lead on SP (issued at t=0; the ACT
                    # queue is busy with the exp-table warmup), then mask
                    # halves at the consumption rate, with V and the second
                    # head-pair deferred to their first use.
                    A.dma_start(out=qks[0][:, 1, 0:512], in_=qk_src(0)[:, 1, 0:512])
                    A.dma_start(out=qks[0][:, 0, 0:QG], in_=qk_src(0)[:, 0, 0:QG])
                    A.dma_start(out=qks[0][:, 1, 512:1024], in_=qk_src(0)[:, 1, 512:1024])
                    Bq.dma_start(out=vps[:, :, 0, 0:HD], in_=v_view[:, :, 0])
                    A.dma_start(out=nm_sb[:, 0, 0:QH], in_=nm_view[:, 0, 0:QH])
                    Bq.dma_start(out=nm_sb[:, 1, 0:QH], in_=nm_view[:, 1, 0:QH])
                    A.dma_start(out=nm_sb[:, 2, 0:QH], in_=nm_view[:, 2, 0:QH])
                    A.dma_start(out=qks[0][:, 1, 1024:2048], in_=qk_src(0)[:, 1, 1024:2048])
                    Bq.dma_start(out=nm_sb[:, 3, 0:QH], in_=nm_view[:, 3, 0:QH])
                    for ks in range(4, KS):
                        (A if ks % 2 == 0 else Bq).dma_start(
                            out=nm_sb[:, ks, 0:QH], in_=nm_view[:, ks, 0:QH]
                        )
                        if ks == 8:
                            A.dma_start(out=qks[0][:, 0, QG:2 * QG],
                                        in_=qk_src(0)[:, 0, QG:2 * QG])
                    # second batch: V heads 1-3, q-group-1 mask halves, pair 1
                    Bq.dma_start(out=vps[:, :, 1, 0:HD], in_=v_view[:, :, 1])
                    for ks in range(KS):
                        (A if ks % 2 == 0 else Bq).dma_start(
                            out=nm_sb[:, ks, QH:2 * QH], in_=nm_view[:, ks, QH:2 * QH]
                        )
                        if ks == 6:
                            Bq.dma_start(out=vps[:, :, 2, 0:HD], in_=v_view[:, :, 2])
                        if ks == 10:
                            A.dma_start(out=qks[1], in_=qk_src(1))
                        if ks == 12:
                            Bq.dma_start(out=vps[:, :, 3, 0:HD], in_=v_view[:, :, 3])
                else:
                    A, Bq = nc.sync, nc.gpsimd
                    A.dma_start(out=qks[0], in_=qk_src(0))
                    for hh in range(HPC):
                        Bq.dma_start(out=vps[:, :, hh, 0:HD], in_=v_view[:, :, hh])
                    for pair in range(1, HPC // 2):
                        A.dma_start(out=qks[pair], in_=qk_src(pair))
                    for ks in range(KS):
                        (A if ks % 2 == 0 else Bq).dma_start(
                            out=nm_sb[:, ks, :], in_=nm_view[:, ks, :]
                        )

                out_asm = opool.tile([128, KS, HPC * HD], BF16)

                def emit_carry(carry):
                    """Last two AV strips (lag-2 emission) + finalize: Z
                    reciprocal, broadcast normalize, and the output DMA once
                    the last head of a q-group completes."""
                    ch, cqg, cpsO, at_tail = carry
                    for i, (cat, cks) in enumerate(at_tail):
                        last = i == len(at_tail) - 1
                        for j in range(NCH):
                            nc.tensor.matmul(
                                cpsO[:, j, 0:HD + 1],
                                lhsT=cat[:, j * 128:(j + 1) * 128],
                                rhs=vps[:, cks, ch, :],
                                # start/stop once per psum bank (4 chunks of
                                # 512B share a 2KB zero region)
                                start=(cks == 0 and j % CPB == 0),
                                stop=last and (j % CPB == CPB - 1 or j == NCH - 1),
                            )
                    rec = spool.tile([128, NCH], F32)
                    nc.vector.reciprocal(rec, cpsO[:, :, HD])
                    final = ch == HPC - 1 and cqg == NQG - 1
                    step = max(NCH // 2, 1)
                    for lo in range(0, NCH, step):
                        hi = min(lo + step, NCH)
                        sq0 = cqg * NCH + lo
                        nc.vector.tensor_mul(
                            out_asm[:, sq0:sq0 + hi - lo, ch * HD:(ch + 1) * HD],
                            cpsO[:, lo:hi, 0:HD],
                            rec[:, lo:hi].to_broadcast([128, hi - lo, HD]),
                        )
                        if ch == HPC - 1:
                            # the final group's DMAs both ride SP: the Pool
                            # SWDGE prep (~1.1us) would sit on the critical
                            # path at the very end of the program
                            eng = nc.gpsimd if (lo > 0 and not final) else nc.sync
                            eng.dma_start(
                                out=out_view[:, sq0:sq0 + hi - lo, :],
                                in_=out_asm[:, sq0:sq0 + hi - lo, :],
                            )

                carry = None
                groups = [(h, qg) for h in range(HPC) for qg in range(NQG)]
                for gi, (h, qg) in enumerate(groups):
                    base = 64 * (h % 2)
                    qt_r = qks[h // 2][:, 0, :]
                    kt_r = qks[h // 2][:, 1, :]
                    q0 = qg * QG
                    # AV lag: during the DMA-racy head a late mask multiply
                    # must not block the in-order PE queue right before the
                    # QK the ACT engine is waiting on; later groups use lag 1
                    # so the end-of-group AV backlog (and the final tail) is
                    # short.
                    lag = LAG if gi < 2 else (1 if gi == len(groups) - 1 else 2)
                    last_g = gi == len(groups) - 1
                    psO = None
                    ats = {}
                    for ks in range(KS):
                        if ks - lag in ats:
                            at2 = ats.pop(ks - lag)
                            if psO is None:
                                psO = psO_pool.tile([128, NCH, 128], F32)
                            for j in range(NCH):
                                nc.tensor.matmul(
                                    psO[:, j, 0:HD + 1],
                                    lhsT=at2[:, j * 128:(j + 1) * 128],
                                    rhs=vps[:, ks - lag, h, :],
                                    start=(ks == lag and j % CPB == 0),
                                    stop=False,
                                )
                        psS = psS_pool.tile([128, QG], F32, tag="psS")
                        for qc in range(NQC):
                            nc.tensor.matmul(
                                psS[:, qc * QC:(qc + 1) * QC],
                                lhsT=kt_r[base:base + HD, ks * 128:(ks + 1) * 128],
                                rhs=qt_r[base:base + HD,
                                         q0 + qc * QC:q0 + (qc + 1) * QC],
                                start=True,
                                stop=True,
                            )
                        if ks == 0 and carry is not None:
                            emit_carry(carry)
                            carry = None
                        at = apool.tile([128, QG], BF16, tag="at")
                        # final strip of the final group: exp+mask in halves
                        # so the tail AV chunks start half an exp earlier
                        # (subtile deps let AV chunks 0-3 run off half 1)
                        nsp = 2 if (last_g and ks == KS - 1 and QG >= 1024) else 1
                        for sp in range(nsp):
                            sl = slice(sp * QG // nsp, (sp + 1) * QG // nsp)
                            nc.scalar.activation(
                                at[:, sl], psS[:, sl],
                                mybir.ActivationFunctionType.Exp,
                                scale=0.125,
                            )
                            nc.vector.tensor_mul(
                                at[:, sl], at[:, sl],
                                nm_sb[:, ks, q0 + sl.start:q0 + sl.stop],
                            )
                        ats[ks] = at
                    if psO is None:
                        psO = psO_pool.tile([128, NCH, 128], F32)
                    tail = sorted(ats.items())
                    carry = (h, qg, psO, [(a, k) for k, a in tail])
                emit_carry(carry)

            for _ in range(reps):
                emit_body()
    nc.compile()
    return nc


_CACHE = {}


def _get_nc():
    if "nc" not in _CACHE:
        _CACHE["nc"] = build_program()
    return _CACHE["nc"]


def make_in_maps(q, k, v, mask, s=S):
    """Shard full inputs into 8 per-core input maps (host-side layout prep)."""
    q = np.asarray(q, dtype=np.float32)
    k = np.asarray(k, dtype=np.float32)
    v = np.asarray(v, dtype=np.float32)
    mask = np.asarray(mask)
    nh = q.shape[-1] // HD
    in_maps = []
    for c in range(NCORES):
        b, g = divmod(c, NCORES // B)
        h0 = HPC * g
        qs = q[b].reshape(s, nh, HD)[:, h0:h0 + HPC, :]      # [s, HPC, 64]
        ks_ = k[b].reshape(s, nh, HD)[:, h0:h0 + HPC, :]
        qkT = np.empty((2, HPC * HD, s), ml_dtypes.bfloat16)
        qkT[0] = qs.transpose(1, 2, 0).reshape(HPC * HD, s)
        qkT[1] = ks_.transpose(1, 2, 0).reshape(HPC * HD, s)
        vc = np.ascontiguousarray(v[b, :, h0 * HD:(h0 + HPC) * HD]).astype(
            ml_dtypes.bfloat16
        )
        nmT = np.ascontiguousarray((~mask[b]).T).astype(ml_dtypes.bfloat16)
        in_maps.append({"qkT": qkT, "v": vc, "nmT": nmT})
    return in_maps


def assemble_out(results, s=S, d=D):
    out = np.empty((B, s, d), np.float32)
    for c in range(NCORES):
        b, g = divmod(c, NCORES // B)
        out[b, :, g * HPC * HD:(g + 1) * HPC * HD] = results[c]["out"]
    return out


def kernel(q, k, v, mask):
    nc = _get_nc()
    in_maps = make_in_maps(q, k, v, mask)
    res = run_bass_kernel_spmd(nc, in_maps, list(range(NCORES))).results
    return assemble_out(res)


# revision 18
# speedup vs baseline: 1.0111x; 1.0111x over previous
# Multi-head attention (B=2, S=2048, D=1024, H=16, head_dim=64) with bool mask,
# sharded across 8 TRN2 NeuronCores: core c -> batch c//4, heads 4*(c%4)..4*(c%4)+3.
#
# Per-core device kernel (scores computed transposed: scoresT[k, q]):
#   scoresT = K @ Q^T                 (PE bf16, lhsT = K^T strip, rhs = Q^T)
#   atp     = exp(scoresT/8)          (ACT exp scale=1/8, psum -> psum bf16)
#   at      = atp * (1-m)T            (DVE mult, psum -> SBUF bf16)
#   out[q,d] += at_chunk^T @ [V|1]    (PE bf16: lhsT = at chunk (stationary),
#                                      rhs = V'[128,65]; col 64 accumulates Z)
#   out     = psO[:, :, 0:64] / Z     (DVE reciprocal + broadcast multiply)
#
# The AV matmul uses the attention chunk as the stationary operand so the
# output lands non-transposed ([q, d] with q on partitions): free size is 65
# instead of 512 per instruction (half the PE cycles of the V-stationary
# form) and the final PE transposes disappear entirely.
#
# Host side (inside kernel()): slice per-core shards, pre-transpose Q/K per
# head ([64, S] head-dim-major, bf16), pre-transpose the inverted mask to
# bf16, reassemble the 8 per-core bf16 outputs into the full f32 output.

import sys

import numpy as np

for _p in ("/opt/trn_rl_repo",):
    if _p not in sys.path:
        sys.path.insert(0, _p)

import ml_dtypes

import concourse.bass as bass  # noqa: F401  (engine types reachable via nc)
import concourse.tile as tile
from concourse import bacc, mybir
from concourse.bass_utils import run_bass_kernel_spmd

F32 = mybir.dt.float32
BF16 = mybir.dt.bfloat16

S = 2048          # sequence length
HD = 64           # head dim
HPC = 4           # heads per core
NCORES = 8
B = 2
H = 16
D = H * HD


def build_program(s=S, reps=1):
    """Build the single-core SPMD program. Returns the compiled Bacc object.

    reps>1 emits the whole body (loads+compute+stores) that many times in one
    NEFF — used to measure device time by wall-clock differencing."""
    nc = bacc.Bacc()

    KS = s // 128            # number of k strips
    QG = 1024 if s >= 1024 else s   # q group width (ACT/DVE instruction width)
    NQG = s // QG            # q groups
    NQC = max(QG // 512, 1)  # 512-wide matmul chunks per q group (psum bank)
    QC = min(512, QG)        # matmul chunk width
    NCH = QG // 128          # 128-wide q chunks per group (AV granularity)
    CPB = 4                  # psO chunks per 2KB psum bank (zero region)
    LAG = min(4, KS)         # AV strips emitted this many strips behind QK

    qkT_d = nc.declare_dram_parameter("qkT", [2, HPC * HD, s], BF16, isOutput=False)
    v_d = nc.declare_dram_parameter("v", [s, HPC * HD], BF16, isOutput=False)
    nmT_d = nc.declare_dram_parameter("nmT", [s, s], BF16, isOutput=False)
    out_d = nc.declare_dram_parameter("out", [s, HPC * HD], BF16, isOutput=True)

    # DRAM views with the k/q axis split into strips of 128 partitions
    nm_view = nmT_d[:].rearrange("(ks p) q -> p ks q", p=128)
    v_view = v_d[:].rearrange("(ks p) (h d) -> p ks h d", p=128, h=HPC)
    out_view = out_d[:].rearrange("(sq p) c -> p sq c", p=128)

    with tile.TileContext(nc) as tc:
        with (
            tc.tile_pool(name="const", bufs=1) as const,
            tc.tile_pool(name="wq", bufs=1) as wq,
            tc.tile_pool(name="attn", bufs=20) as apool,
            tc.tile_pool(name="stat", bufs=4) as spool,
            tc.tile_pool(name="oasm", bufs=1) as opool,
            tc.tile_pool(name="psS", bufs=2, space="PSUM") as psS_pool,
            tc.tile_pool(name="psO", bufs=2, space="PSUM") as psO_pool,
        ):
            # Preload the exp table (emitted before any real exp; runs while
            # the first DMAs stream).
            warm = const.tile([128, 1], F32)
            nc.vector.memset(warm, 0.0)
            nc.scalar.activation(warm, warm, mybir.ActivationFunctionType.Exp)

            # Warm the PE HAM clock gate while input DMAs run: ~3us of dummy
            # matmuls so the first real QKs run at 2.4GHz.
            zb = const.tile([128, 128], BF16)
            nc.vector.memset(zb, 0.0)
            for _ in range(24):
                wmm = psS_pool.tile([128, QG], F32, tag="psS")
                nc.tensor.matmul(
                    wmm[:, :128], lhsT=zb[0:64, :], rhs=zb[0:64, :],
                    start=True, stop=True,
                )

            def qk_src(pair):
                return qkT_d[:, 128 * pair:128 * pair + 128, :].rearrange(
                    "t p s -> p t s"
                )

            def emit_body():
                # Q^T / K^T head pairs: [128, s] (head 2p on partitions 0-63,
                # head 2p+1 on partitions 64-127).
                qks = []
                for pair in range(HPC // 2):
                    qk = wq.tile([128, 2, s], BF16, tag=f"qkT{pair}")
                    qks.append(qk)
                # V' staging: [128, ks, h, 65] with a ones column at 64 so the
                # AV matmul's 65th output column accumulates the softmax
                # denominator Z. V lands via interleaved DMA; the ones column
                # is memset once (disjoint subtile, no dependency on the DMA).
                vps = wq.tile([128, KS, HPC, HD + 1], BF16, tag="vps")
                nm_sb = wq.tile([128, KS, s], BF16, tag="nm")
                nc.vector.memset(vps[:, :, :, HD:HD + 1], 1.0)

                # DMA choreography (s=2048): two queues only — SP (nc.sync)
                # and Pool SWDGE (nc.gpsimd) — so the ACT and DVE sequencers
                # are never blocked behind a DMA wait. Ordered by first use:
                # K strips + first Q group first, mask halves interleaved,
                # V early (AV matmuls sit in the in-order PE queue).
                QH = QG  # nm half width
                if s == 2048:
                    A, Bq = nc.sync, nc.gpsimd
                    # The model's DMA device is effectively serial, so the
                    # ordering across the queues is what matters: the
                    # first-QK inputs lead on SP (issued at t=0; the ACT
                    # queue is busy with the exp-table warmup), then mask
                    # halves at the consumption rate, with V and the second
                    # head-pair deferred to their first use.
                    A.dma_start(out=qks[0][:, 1, 0:512], in_=qk_src(0)[:, 1, 0:512])
                    A.dma_start(out=qks[0][:, 0, 0:QG], in_=qk_src(0)[:, 0, 0:QG])
                    A.dma_start(out=qks[0][:, 1, 512:1024], in_=qk_src(0)[:, 1, 512:1024])
                    Bq.dma_start(out=vps[:, :, 0, 0:HD], in_=v_view[:, :, 0])
                    A.dma_start(out=nm_sb[:, 0, 0:QH], in_=nm_view[:, 0, 0:QH])
                    Bq.dma_start(out=nm_sb[:, 1, 0:QH], in_=nm_view[:, 1, 0:QH])
                    A.dma_start(out=nm_sb[:, 2, 0:QH], in_=nm_view[:, 2, 0:QH])
                    A.dma_start(out=qks[0][:, 1, 1024:2048], in_=qk_src(0)[:, 1, 1024:2048])
                    Bq.dma_start(out=nm_sb[:, 3, 0:QH], in_=nm_view[:, 3, 0:QH])
                    for ks in range(4, KS):
                        (A if ks % 2 == 0 else Bq).dma_start(
                            out=nm_sb[:, ks, 0:QH], in_=nm_view[:, ks, 0:QH]
                        )
                        if ks == 8:
                            A.dma_start(out=qks[0][:, 0, QG:2 * QG],
                                        in_=qk_src(0)[:, 0, QG:2 * QG])
                    # second batch: q-group-1 mask halves; V heads 1-3 and
                    # the second head pair are needed only from groups 2/4/6
                    # (~33/66/100us) so they go after all the mask halves.
                    Bq.dma_start(out=vps[:, :, 1, 0:HD], in_=v_view[:, :, 1])
                    for ks in range(KS):
                        (A if ks % 2 == 0 else Bq).dma_start(
                            out=nm_sb[:, ks, QH:2 * QH], in_=nm_view[:, ks, QH:2 * QH]
                        )
                    A.dma_start(out=qks[1], in_=qk_src(1))
                    Bq.dma_start(out=vps[:, :, 2, 0:HD], in_=v_view[:, :, 2])
                    Bq.dma_start(out=vps[:, :, 3, 0:HD], in_=v_view[:, :, 3])
                else:
                    A, Bq = nc.sync, nc.gpsimd
                    A.dma_start(out=qks[0], in_=qk_src(0))
                    for hh in range(HPC):
                        Bq.dma_start(out=vps[:, :, hh, 0:HD], in_=v_view[:, :, hh])
                    for pair in range(1, HPC // 2):
                        A.dma_start(out=qks[pair], in_=qk_src(pair))
                    for ks in range(KS):
                        (A if ks % 2 == 0 else Bq).dma_start(
                            out=nm_sb[:, ks, :], in_=nm_view[:, ks, :]
                        )

                out_asm = opool.tile([128, KS, HPC * HD], BF16)

                def emit_carry(carry):
                    """Last two AV strips (lag-2 emission) + finalize: Z
                    reciprocal, broadcast normalize, and the output DMA once
                    the last head of a q-group completes."""
                    ch, cqg, cpsO, at_tail = carry
                    for i, (cat, cks) in enumerate(at_tail):
                        last = i == len(at_tail) - 1
                        for j in range(NCH):
                            nc.tensor.matmul(
                                cpsO[:, j, 0:HD + 1],
                                lhsT=cat[:, j * 128:(j + 1) * 128],
                                rhs=vps[:, cks, ch, :],
                                # start/stop once per psum bank (4 chunks of
                                # 512B share a 2KB zero region)
                                start=(cks == 0 and j % CPB == 0),
                                stop=last and (j % CPB == CPB - 1 or j == NCH - 1),
                            )
                    rec = spool.tile([128, NCH], F32)
                    nc.vector.reciprocal(rec, cpsO[:, :, HD])
                    final = ch == HPC - 1 and cqg == NQG - 1
                    step = max(NCH // 2, 1)
                    for lo in range(0, NCH, step):
                        hi = min(lo + step, NCH)
                        sq0 = cqg * NCH + lo
                        nc.vector.tensor_mul(
                            out_asm[:, sq0:sq0 + hi - lo, ch * HD:(ch + 1) * HD],
                            cpsO[:, lo:hi, 0:HD],
                            rec[:, lo:hi].to_broadcast([128, hi - lo, HD]),
                        )
                        if ch == HPC - 1:
                            # the final group's DMAs both ride SP: the Pool
                            # SWDGE prep (~1.1us) would sit on the critical
                            # path at the very end of the program
                            eng = nc.gpsimd if (lo > 0 and not final) else nc.sync
                            eng.dma_start(
                                out=out_view[:, sq0:sq0 + hi - lo, :],
                                in_=out_asm[:, sq0:sq0 + hi - lo, :],
                            )

                carry = None
                groups = [(h, qg) for h in range(HPC) for qg in range(NQG)]
                for gi, (h, qg) in enumerate(groups):
                    base = 64 * (h % 2)
                    qt_r = qks[h // 2][:, 0, :]
                    kt_r = qks[h // 2][:, 1, :]
                    q0 = qg * QG
                    # AV lag: during the DMA-racy head a late mask multiply
                    # must not block the in-order PE queue right before the
                    # QK the ACT engine is waiting on; later groups use lag 1
                    # so the end-of-group AV backlog (and the final tail) is
                    # short.
                    lag = LAG if gi < 2 else (1 if gi == len(groups) - 1 else 2)
                    last_g = gi == len(groups) - 1
                    psO = None
                    ats = {}
                    for ks in range(KS):
                        if ks - lag in ats:
                            at2 = ats.pop(ks - lag)
                            if psO is None:
                                psO = psO_pool.tile([128, NCH, 128], F32)
                            for j in range(NCH):
                                nc.tensor.matmul(
                                    psO[:, j, 0:HD + 1],
                                    lhsT=at2[:, j * 128:(j + 1) * 128],
                                    rhs=vps[:, ks - lag, h, :],
                                    start=(ks == lag and j % CPB == 0),
                                    stop=False,
                                )
                        psS = psS_pool.tile([128, QG], F32, tag="psS")
                        for qc in range(NQC):
                            nc.tensor.matmul(
                                psS[:, qc * QC:(qc + 1) * QC],
                                lhsT=kt_r[base:base + HD, ks * 128:(ks + 1) * 128],
                                rhs=qt_r[base:base + HD,
                                         q0 + qc * QC:q0 + (qc + 1) * QC],
                                start=True,
                                stop=True,
                            )
                        if ks == 0 and carry is not None:
                            emit_carry(carry)
                            carry = None
                        at = apool.tile([128, QG], BF16, tag="at")
                        # final strip of the final group: exp+mask in halves
                        # so the tail AV chunks start half an exp earlier
                        # (subtile deps let AV chunks 0-3 run off half 1)
                        nsp = 2 if (last_g and ks == KS - 1 and QG >= 1024) else 1
                        for sp in range(nsp):
                            sl = slice(sp * QG // nsp, (sp + 1) * QG // nsp)
                            nc.scalar.activation(
                                at[:, sl], psS[:, sl],
                                mybir.ActivationFunctionType.Exp,
                                scale=0.125,
                            )
                            nc.vector.tensor_mul(
                                at[:, sl], at[:, sl],
                                nm_sb[:, ks, q0 + sl.start:q0 + sl.stop],
                            )
                        ats[ks] = at
                    if psO is None:
                        psO = psO_pool.tile([128, NCH, 128], F32)
                    tail = sorted(ats.items())
                    carry = (h, qg, psO, [(a, k) for k, a in tail])
                emit_carry(carry)

            for _ in range(reps):
                emit_body()
    nc.compile()
    return nc


_CACHE = {}


def _get_nc():
    if "nc" not in _CACHE:
        _CACHE["nc"] = build_program()
    return _CACHE["nc"]


def make_in_maps(q, k, v, mask, s=S):
    """Shard full inputs into 8 per-core input maps (host-side layout prep)."""
    q = np.asarray(q, dtype=np.float32)
    k = np.asarray(k, dtype=np.float32)
    v = np.asarray(v, dtype=np.float32)
    mask = np.asarray(mask)
    nh = q.shape[-1] // HD
    in_maps = []
    for c in range(NCORES):
        b, g = divmod(c, NCORES // B)
        h0 = HPC * g
        qs = q[b].reshape(s, nh, HD)[:, h0:h0 + HPC, :]      # [s, HPC, 64]
        ks_ = k[b].reshape(s, nh, HD)[:, h0:h0 + HPC, :]
        qkT = np.empty((2, HPC * HD, s), ml_dtypes.bfloat16)
        qkT[0] = qs.transpose(1, 2, 0).reshape(HPC * HD, s)
        qkT[1] = ks_.transpose(1, 2, 0).reshape(HPC * HD, s)
        vc = np.ascontiguousarray(v[b, :, h0 * HD:(h0 + HPC) * HD]).astype(
            ml_dtypes.bfloat16
        )
        nmT = np.ascontiguousarray((~mask[b]).T).astype(ml_dtypes.bfloat16)
        in_maps.append({"qkT": qkT, "v": vc, "nmT": nmT})
    return in_maps


def assemble_out(results, s=S, d=D):
    out = np.empty((B, s, d), np.float32)
    for c in range(NCORES):
        b, g = divmod(c, NCORES // B)
        out[b, :, g * HPC * HD:(g + 1) * HPC * HD] = results[c]["out"]
    return out


def kernel(q, k, v, mask):
    nc = _get_nc()
    in_maps = make_in_maps(q, k, v, mask)
    res = run_bass_kernel_spmd(nc, in_maps, list(range(NCORES))).results
    return assemble_out(res)
